# revision 1
# baseline (speedup 1.0000x reference)
"""Trainium2 Bass kernel for nn_DecoderAttention_38817914421501.

Multi-head attention: out = softmax(Q@K^T / sqrt(64)) @ V, per (batch, head).
N=8, L=2048, D=64, H=4, head_dim=16.

Sharding: data-parallel over batch N across the 8 NeuronCores (one batch
element per core). Inside each core:
  - Q, K are transposed on-chip (PE transpose) to [head_dim, L] bf16 layout.
  - scores^T[k, q] = K_h^T.T @ Q_h^T computed per head via TensorE
    (contraction = head_dim on the partition axis), fp32 in PSUM.
  - exp((scores)/8) on ScalarE (ACT) reading PSUM, writing bf16 SBUF.
  - PV: out_aug^T[d, q] accumulated over k-chunks with lhsT = [V_h | 1]
    (the appended ones-column makes the softmax denominator a free 17th row).
  - Per 128-query chunk: PE-transpose out_aug^T -> [q, 17], reciprocal of
    column 16, scale columns 0..15, assemble [128, 64] and DMA out.
"""

import os
import sys

import numpy as np

for _p in ("/opt/trn_rl_repo", "/root/.axon_site/_ro/trn_rl_repo"):
    if _p not in sys.path and os.path.isdir(_p):
        sys.path.append(_p)

import concourse.bass as bass
import concourse.bacc as bacc
import concourse.tile as tile
from concourse import mybir
from concourse.bass_utils import run_bass_kernel_spmd
from concourse.masks import make_identity

N, L, D, H, HD = 8, 2048, 64, 4, 16
NKC = L // 128          # 16 k-chunks of 128 keys
NQC = L // 512          # 4 q-chunks of 512 queries
SCALE = 1.0 / np.sqrt(np.float32(D))  # 1/8

F32 = mybir.dt.float32
BF16 = mybir.dt.bfloat16


def build_nc():
    nc = bacc.Bacc("TRN2", target_bir_lowering=False, debug=False)

    q_d = nc.dram_tensor("q", [L, D], F32, kind="ExternalInput").ap()
    k_d = nc.dram_tensor("k", [L, D], F32, kind="ExternalInput").ap()
    v_d = nc.dram_tensor("v", [L, D], F32, kind="ExternalInput").ap()
    o_d = nc.dram_tensor("out", [L, D], F32, kind="ExternalOutput").ap()

    with tile.TileContext(nc) as tc:
        with (
            tc.tile_pool(name="singles", bufs=1) as singles,
            tc.tile_pool(name="stage", bufs=3) as stage_pool,
            tc.tile_pool(name="ex", bufs=8) as ex_pool,
            tc.tile_pool(name="outp", bufs=3) as out_pool,
            tc.tile_pool(name="small", bufs=8) as small_pool,
            tc.tile_pool(name="pvs", bufs=1) as pvs_pool,
            tc.tile_pool(name="scps", bufs=4, space="PSUM") as sc_pool,
            tc.tile_pool(name="pvps", bufs=4, space="PSUM") as pv_pool,
        ):
            ident = singles.tile([128, 128], F32)
            make_identity(nc, ident)

            # qt/kt: [128, L] bf16; head h occupies partitions 32h..32h+15
            # (32-strided so each head sits in its own PE row-tile strip).
            qt = singles.tile([128, L], BF16)
            kt = singles.tile([128, L], BF16)
            # vaug: [128, kc, h, 17] bf16; col 16 of each (kc, h) block is 1.0
            vaug = singles.tile([128, NKC, H, HD + 1], BF16)
            nc.gpsimd.memset(vaug, 1.0)

            # ---- Phase A: load + transpose Q, K; build V_aug ----
            for (src, dst) in ((q_d, qt), (k_d, kt)):
                for t in range(NKC):
                    # zero-padded stage: col 32h+d holds src[:, 16h+d]; the
                    # pad columns make the transpose land heads at 32h+d rows.
                    st = stage_pool.tile([128, 128], F32, tag="stage")
                    nc.gpsimd.memset(st, 0.0)
                    nc.sync.dma_start(
                        out=st.rearrange("p (h x) -> p h x", h=H)[:, :, 0:HD],
                        in_=src[t * 128:(t + 1) * 128, :]
                        .rearrange("p (h d) -> p h d", h=H),
                    )
                    tp = sc_pool.tile([128, 128], F32, tag="sc")
                    nc.tensor.transpose(tp, st, ident)
                    nc.vector.tensor_copy(dst[:, t * 128:(t + 1) * 128], tp)

            for t in range(NKC):
                st = stage_pool.tile([128, D], F32, tag="stage")
                nc.sync.dma_start(out=st, in_=v_d[t * 128:(t + 1) * 128, :])
                nc.vector.tensor_copy(
                    vaug[:, t, :, 0:HD],
                    st.rearrange("p (h d) -> p h d", h=H),
                )

            # ---- Phase B: attention main loop ----
            for qc in range(NQC):
                qs = qc * 512
                pv = [pv_pool.tile([HD + 1, 512], F32, tag="pv", name=f"pv{h}")
                      for h in range(H)]
                for kc in range(NKC):
                    for h in range(H):
                        sc = sc_pool.tile([128, 512], F32, tag="sc")
                        nc.tensor.matmul(
                            sc,
                            lhsT=kt[32 * h:32 * h + HD, kc * 128:(kc + 1) * 128],
                            rhs=qt[32 * h:32 * h + HD, qs:qs + 512],
                            start=True, stop=True,
                            tile_position=(32 * h, 0),
                        )
                        ex = ex_pool.tile([128, 512], BF16, tag="ex")
                        nc.scalar.activation(
                            ex, sc, mybir.ActivationFunctionType.Exp,
                            scale=float(SCALE),
                        )
                        nc.tensor.matmul(
                            pv[h],
                            lhsT=vaug[:, kc, h, :],
                            rhs=ex,
                            start=(kc == 0), stop=(kc == NKC - 1),
                        )

                # ---- tail: normalize + transpose to [q, d] and store ----
                pvs = pvs_pool.tile([HD + 1, H, 512], F32, tag="pvs")
                for h in range(H):
                    nc.vector.tensor_copy(pvs[:, h, :], pv[h])
                for s in range(4):
                    ob = out_pool.tile([128, D], F32, tag="ob")
                    for h in range(H):
                        tt = sc_pool.tile([128, HD + 1], F32, tag="sc")
                        nc.tensor.transpose(
                            tt,
                            pvs[:, h, s * 128:(s + 1) * 128],
                            ident[0:HD + 1, 0:HD + 1],
                        )
                        r = small_pool.tile([128, 1], F32, tag="r")
                        nc.vector.reciprocal(r, tt[:, HD:HD + 1])
                        nc.vector.tensor_scalar_mul(
                            ob[:, 16 * h:16 * (h + 1)], tt[:, 0:HD], r,
                        )
                    nc.sync.dma_start(
                        out=o_d[qs + s * 128: qs + (s + 1) * 128, :], in_=ob,
                    )

    return nc


_NC = None
last_exec_time_ns = None
last_results = None


def kernel(query, key, value):
    global _NC, last_exec_time_ns, last_results
    query = np.asarray(query, dtype=np.float32)
    key = np.asarray(key, dtype=np.float32)
    value = np.asarray(value, dtype=np.float32)
    assert query.shape == (N, L, D)

    if _NC is None:
        _NC = build_nc()
        _NC.finalize()

    in_maps = [
        {
            "q": np.ascontiguousarray(query[i]),
            "k": np.ascontiguousarray(key[i]),
            "v": np.ascontiguousarray(value[i]),
        }
        for i in range(N)
    ]
    res = run_bass_kernel_spmd(
        _NC, in_maps, core_ids=list(range(N)),
        trace=bool(int(os.environ.get("KERNEL_TRACE", "0"))),
    )
    last_results = res
    last_exec_time_ns = res.exec_time_ns
    out = np.stack([res.results[i]["out"] for i in range(N)], axis=0)
    return out

